# revision 6
# baseline (speedup 1.0000x reference)
"""GTN (graph transformer network) forward on 8 Trainium2 cores.

Math (mirrors the reference; normalizations folded, matmuls re-associated):
  A[t]  = dense adjacency from edge lists              (host, bincount)
  A1 = softmax(w_l0_c1) . A ; A2 = softmax(w_l0_c2) . A ; A3 = softmax(w_l1_c1) . A
  U  = A1 @ A2 @ A3  (never materialized!)
  The output only needs U @ XW (XW = X @ gcn_w, [N,128]) and rowsum(U):
    U @ XW     = A1 @ (A2 @ (A3 @ XW))      three [N,N]@[N,128] products
    rowsum(U)  = A1 @ (A2 @ rowsum(A3))     two GEMVs, done on host
  and only at the unique target_x rows, so stage 1 computes just those
  (~900 of 4096) rows.  This is ~25x fewer FLOPs than forming A1@A2@A3.
  Row-normalizing only at the end is exact: row scaling commutes through
  matmul and all entries are >= 0 (zero rows stay zero either way).
  y = relu(Z/rowsum + b) -> channel concat -> target gather -> linear (host).

Sharding: 2 channels x 4 row-blocks = 8 cores.  Core ci handles channel
ci//4, rows (ci%4)*1024 ... +1024 (stages 3,2) and a quarter of the unique
target rows (stage 1).  After stages 3 and 2 an AllGather over the 4-core
channel group rebuilds the full [4096,128] operand for the next stage.
Device inputs are column slabs of the *transposed* combos (built transposed
on host for free by swapping src/dst in the bincount), bf16:
  matmul(out[q,d], lhsT=slabT chunk [128k x 128q], rhs chunk [128k x 128d])
k-outer over the contraction so all row-tiles accumulate in parallel PSUM
banks and the first matmul fires as soon as the first slab piece lands.
"""

import os
import time
import numpy as np
from contextlib import ExitStack

NUM_EDGE = 5
C = 2
N = 4096
W_IN = 512
W_OUT = 128
NCORES = 8
P = 128
NGRP = 4                  # cores per channel group
RLOC = N // NGRP          # 1024 rows per core (stages 3, 2)
NK = N // P               # 32 contraction chunks
NM = RLOC // P            # 8 output row tiles per core
NTGT = 256                # padded unique-target rows per core (stage 1)
NM1 = NTGT // P
DOUT = W_OUT              # 128
NPIECE = 8                # DMA pieces per slab
KPP = NK // NPIECE        # k-chunks per piece
GROUPS = [[0, 1, 2, 3], [4, 5, 6, 7]]

_NC_CACHE = {}
LAST_EXEC_NS = None
LAST_RESULTS = None


def _build_nc():
    import concourse.tile as tile
    from concourse import bacc, mybir

    nc = bacc.Bacc("TRN2", target_bir_lowering=False, debug=False,
                   num_devices=NCORES)
    f32 = mybir.dt.float32
    bf16 = mybir.dt.bfloat16

    # lK[j, i] = A_K^T[c][j, rows[i]]  (column slab of transposed combo)
    l3 = nc.dram_tensor("l3", [N, RLOC], bf16, kind="ExternalInput").ap()
    l2 = nc.dram_tensor("l2", [N, RLOC], bf16, kind="ExternalInput").ap()
    l1 = nc.dram_tensor("l1", [N, NTGT], bf16, kind="ExternalInput").ap()
    xw = nc.dram_tensor("xw", [N, DOUT], bf16, kind="ExternalInput").ap()
    z = nc.dram_tensor("z", [NTGT, DOUT], f32, kind="ExternalOutput").ap()

    with tile.TileContext(nc) as tc, ExitStack() as ctx:
        xwp = ctx.enter_context(tc.tile_pool(name="xwp", bufs=1))
        slabp = ctx.enter_context(tc.tile_pool(name="slabp", bufs=2))
        s1p = ctx.enter_context(tc.tile_pool(name="s1p", bufs=1))
        yfp = ctx.enter_context(tc.tile_pool(name="yfp", bufs=2))
        outp = ctx.enter_context(tc.tile_pool(name="outp", bufs=4))
        psp = ctx.enter_context(tc.tile_pool(name="psp", bufs=8, space="PSUM"))
        dramp = ctx.enter_context(tc.tile_pool(name="dramp", bufs=1,
                                               space="DRAM"))

        # xw_sb[p, k*DOUT + d] = xw[P*k + p, d]
        xw_sb = xwp.tile([P, NK * DOUT], bf16, tag="xw")
        nc.gpsimd.dma_start(
            xw_sb[:].rearrange("p (k d) -> p k d", k=NK),
            xw.rearrange("(k p) d -> p k d", p=P))

        # tiny warm-up AllGather: completes during the slab stream, so the
        # CC pipeline is hot when the real gathers arrive (cold-start was
        # measured at ~16us doorbell->exec vs ~5us warm)
        warm_in = dramp.tile([1, 64], bf16, tag="warm_in")
        warm_out = dramp.tile([NGRP, 64], bf16, tag="warm_out")
        nc.gpsimd.dma_start(warm_in[:], xw[0:1, 0:64])
        nc.gpsimd.collective_compute(
            "AllGather", mybir.AluOpType.bypass,
            replica_groups=GROUPS,
            ins=[warm_in.opt()], outs=[warm_out.opt()])

        y3loc = dramp.tile([RLOC, DOUT], bf16, tag="y3loc")
        y3full = dramp.tile([N, DOUT], bf16, tag="y3full")
        y2loc = dramp.tile([RLOC, DOUT], bf16, tag="y2loc")
        y2full = dramp.tile([N, DOUT], bf16, tag="y2full")

        def load_slab(slab_dram, width, pool, tag):
            # sb[p, k*width + i] = slab[P*k + p, i]; 2KB contiguous runs.
            # NPIECE pieces so downstream matmuls start on piece 0.
            # All bulk slab loads share the scalar engine's hardware queue:
            # in-queue ordering streams them in consumption order.
            sb = pool.tile([P, NK * width], bf16, tag=tag)
            kw = KPP * width
            for pc in range(NPIECE):
                nc.scalar.dma_start(
                    sb[:, pc * kw:(pc + 1) * kw]
                      .rearrange("p (k i) -> p k i", k=KPP),
                    slab_dram[pc * KPP * P:(pc + 1) * KPP * P, :]
                      .rearrange("(k p) i -> p k i", p=P))
            return sb

        def stage(slab_sb, nm, rhs_sb, out_dram, out_dtype, sname):
            width = nm * P
            accs = [psp.tile([P, DOUT], f32, tag="acc",
                             name=f"acc_{sname}_{m}") for m in range(nm)]
            # k-outer while pieces stream in; the last piece goes m-outer so
            # acc stops stagger and the copies overlap the remaining matmuls
            for k in range(NK - KPP):
                for m in range(nm):
                    nc.tensor.matmul(
                        accs[m][:],
                        slab_sb[:, k * width + m * P: k * width + (m + 1) * P],
                        rhs_sb[:, k * DOUT:(k + 1) * DOUT],
                        start=(k == 0), stop=False,
                        skip_group_check=True)
            ot = outp.tile([P, nm * DOUT], out_dtype, tag="out",
                           name=f"out_{sname}")
            for m in range(nm):
                for k in range(NK - KPP, NK):
                    nc.tensor.matmul(
                        accs[m][:],
                        slab_sb[:, k * width + m * P: k * width + (m + 1) * P],
                        rhs_sb[:, k * DOUT:(k + 1) * DOUT],
                        start=False, stop=(k == NK - 1),
                        skip_group_check=True)
                nc.vector.tensor_copy(ot[:, m * DOUT:(m + 1) * DOUT],
                                      accs[m][:])
            nc.gpsimd.dma_start(
                out_dram.rearrange("(m p) d -> p m d", p=P),
                ot[:].rearrange("p (m d) -> p m d", m=nm))

        def gather(yloc, yfull):
            nc.gpsimd.collective_compute(
                "AllGather", mybir.AluOpType.bypass,
                replica_groups=GROUPS,
                ins=[yloc.opt()], outs=[yfull.opt()])
            yf_sb = yfp.tile([P, NK * DOUT], bf16, tag="yf")
            nc.gpsimd.dma_start(
                yf_sb[:].rearrange("p (k d) -> p k d", k=NK),
                yfull.rearrange("(k p) d -> p k d", p=P))
            return yf_sb

        slab3 = load_slab(l3, RLOC, slabp, "slab")
        slab2 = load_slab(l2, RLOC, slabp, "slab")
        slab1 = load_slab(l1, NTGT, s1p, "slab1")

        stage(slab3, NM, xw_sb, y3loc, bf16, "s3")
        y3f_sb = gather(y3loc, y3full)
        stage(slab2, NM, y3f_sb, y2loc, bf16, "s2")
        y2f_sb = gather(y2loc, y2full)
        stage(slab1, NM1, y2f_sb, z, f32, "s1")

    nc.compile()
    return nc


def _get_nc():
    if "nc" not in _NC_CACHE:
        _NC_CACHE["nc"] = _build_nc()
    return _NC_CACHE["nc"]


def _softmax_rows(w):
    w = np.asarray(w, np.float32)
    e = np.exp(w - w.max(axis=1, keepdims=True))
    return (e / e.sum(axis=1, keepdims=True)).astype(np.float32)


def _install_ntff_hook():
    """Recreate antenv.axon_hooks if the image lacks it (profiling only)."""
    import sys
    import types
    try:
        from antenv.axon_hooks import get_axon_ntff_profile_hook  # noqa: F401
        return
    except ImportError:
        pass
    try:
        from trn_agent_boot.trn_boot import _ntff_profile_via_ctypes
        import antenv
        mod = types.ModuleType("antenv.axon_hooks")
        state = {"h": None}
        mod.set_axon_ntff_profile_hook = lambda h: state.__setitem__("h", h)
        mod.get_axon_ntff_profile_hook = lambda: state["h"]
        sys.modules["antenv.axon_hooks"] = mod
        antenv.axon_hooks = mod
        mod.set_axon_ntff_profile_hook(
            _ntff_profile_via_ctypes("/opt/axon/libaxon_pjrt.so"))
    except Exception:
        pass


def kernel(edge_index, edge_value, X, target_x, w_l0_c1, w_l0_c2, w_l1_c1,
           gcn_w, gcn_b, lin_w, lin_b):
    global LAST_EXEC_NS, LAST_RESULTS
    import ml_dtypes
    from concourse.bass_utils import run_bass_kernel_spmd

    bf16 = ml_dtypes.bfloat16

    # transposed dense adjacency stack [NUM_EDGE, N*N] (dst-major == A^T),
    # duplicate edges summed
    src = np.asarray(edge_index[:, 0], np.int64)
    dst = np.asarray(edge_index[:, 1], np.int64)
    ATf = np.empty((NUM_EDGE, N * N), np.float32)
    for t in range(NUM_EDGE):
        flat = dst[t] * N + src[t]
        ATf[t] = np.bincount(flat, weights=np.asarray(edge_value[t], np.float64),
                             minlength=N * N).astype(np.float32)

    def combo(w):
        f = _softmax_rows(w)                 # [C, NUM_EDGE]
        return (f @ ATf).reshape(C, N, N)    # transposed combos [C, N, N]

    A1T = combo(w_l0_c1)
    A2T = combo(w_l0_c2)
    A3T = combo(w_l1_c1)
    ATf = None  # free

    # rowsum(U) = A1 @ (A2 @ rowsum(A3)), as cheap host GEMVs on the
    # transposed combos: A @ v == v @ A^T.
    s = np.empty((C, N), np.float32)
    for c in range(C):
        v = A3T[c].sum(axis=0)               # rowsum(A3_c)
        s[c] = (v @ A2T[c]) @ A1T[c]

    XW = np.asarray(X, np.float32) @ np.asarray(gcn_w, np.float32)  # [N, 128]
    xwb = XW.astype(bf16)

    # unique target rows, split over the 4 ranks of each channel group
    tgt = np.asarray(target_x, np.int64)
    u, inv = np.unique(tgt, return_inverse=True)
    nu = len(u)
    assert nu <= NGRP * NTGT, nu
    per = [u[r * NTGT:(r + 1) * NTGT] for r in range(NGRP)]

    A1Tb = A1T.astype(bf16)
    A2Tb = A2T.astype(bf16)
    A3Tb = A3T.astype(bf16)
    A1T = A2T = A3T = None

    in_maps = []
    for ci in range(NCORES):
        c, r = divmod(ci, NGRP)
        sl = slice(r * RLOC, (r + 1) * RLOC)
        l1c = np.zeros((N, NTGT), bf16)
        if len(per[r]):
            l1c[:, :len(per[r])] = A1Tb[c][:, per[r]]
        in_maps.append({
            "l1": l1c,
            "l2": np.ascontiguousarray(A2Tb[c][:, sl]),
            "l3": np.ascontiguousarray(A3Tb[c][:, sl]),
            "xw": xwb,
        })

    nc = _get_nc()
    _install_ntff_hook()
    trace = os.environ.get("GTN_TRACE", "1") != "0"
    t0 = time.time()
    res = None
    if trace:
        try:
            res = run_bass_kernel_spmd(nc, in_maps, list(range(NCORES)),
                                       trace=True,
                                       trace_cores=list(range(NCORES)))
        except Exception as e:
            import traceback
            traceback.print_exc()
            print(f"[kernel] trace run failed ({e!r}); retrying untraced")
            res = None
    if res is None:
        res = run_bass_kernel_spmd(nc, in_maps, list(range(NCORES)),
                                   trace=False)
    wall_ns = int((time.time() - t0) * 1e9)
    LAST_EXEC_NS = res.exec_time_ns if res.exec_time_ns else wall_ns
    LAST_RESULTS = res

    # z rows: cores c*4+r carry unique-target rows per[r] of channel c
    Zu = np.empty((C, nu, DOUT), np.float32)
    for c in range(C):
        for r in range(NGRP):
            blk = per[r]
            if len(blk):
                Zu[c, r * NTGT: r * NTGT + len(blk)] = \
                    res.results[c * NGRP + r]["z"][:len(blk)]
    su = s[:, u]                                             # [C, nu]
    with np.errstate(divide="ignore", invalid="ignore"):
        sinv = np.where(su == 0, 0.0, 1.0 / su).astype(np.float32)
    Hn = Zu * sinv[:, :, None]                               # [C, nu, 128]
    Xc = np.maximum(Hn + np.asarray(gcn_b, np.float32)[None, None, :], 0.0)
    X_ = Xc.transpose(1, 0, 2).reshape(nu, C * W_OUT)        # [nu, 256]
    y = X_[inv] @ np.asarray(lin_w, np.float32)
    y = y + np.asarray(lin_b, np.float32)
    return y.astype(np.float32)
